# revision 11
# baseline (speedup 1.0000x reference)
"""Trainium2 Bass kernel for NeuralTensorLayer (order-1/2/3 polynomial layer).

    out[b,l] = bias[l] + sum_i X[b,i] W1[i,l]
             + sum_ij X[b,i] X[b,j] W2[i,j,l]
             + sum_ijk X[b,i] X[b,j] X[b,k] W3[i,j,k,l]

with B=32768, D=K=32, data-parallel over 8 NeuronCores (4096 rows each).

Strategy (per core):
  * Exploit (i,j) symmetry: only the 528 pairs i<=j are needed against
    host-symmetrized weights W3s[ij,k,l] = W3[i,j,k,l]+W3[j,i,k,l] (i<j),
    cutting the dominant matmul contraction from 1024 -> 528.
  * Per 512-row supertile: transpose X via DMA-xbar (bf16), expand X^T rows
    to pair rows with two constant 0/1 selection matmuls on the PE
    (exp[p,b]=X[i_p,b], rep[p,b]=X[j_p,b]), multiply on the DVE to get
    Z^T[p,b] = X_i X_j in bf16.
  * Main matmul (bf16, fp32 PSUM accumulation): T3[b, l*32+k] = sum_p Z^T W3s
    plus a separate [128,32] PSUM region out_low = Z@W2s + X@W1.
  * Post: stage T3 to SBUF bf16 (ScalarE), U = T3 * X_k broadcast (DVE 2x),
    reduce over k (DVE), add out_low, DMA out.  bias added on host.
"""

import numpy as np
import ml_dtypes
from contextlib import ExitStack

import concourse.bass as bass
import concourse.bacc as bacc
import concourse.tile as tile
from concourse import mybir
from concourse import bass_utils

BF16 = ml_dtypes.bfloat16

B, D, KOUT = 32768, 32, 32
NCORES = 8
BLOC = B // NCORES          # 4096 rows per core
P = 128                     # rows per tile
SUPER = 4                   # tiles per supertile
NSUPER = BLOC // (P * SUPER)  # 8
NPAIRS = D * (D + 1) // 2   # 528
CHUNKS = [128, 128, 128, 128, 16]
N3 = 1024                   # T3 columns (l*32 + k)

PAIRS = [(i, j) for i in range(D) for j in range(i, D)]
I_P = np.array([p[0] for p in PAIRS], np.int32)
J_P = np.array([p[1] for p in PAIRS], np.int32)

F32 = mybir.dt.float32
BF = mybir.dt.bfloat16


def _pack_weights(W1, W2, W3):
    W1 = np.asarray(W1, np.float64)
    W2 = np.asarray(W2, np.float64)
    W3 = np.asarray(W3, np.float64)
    # rows: 528 pairs; cols: 1024 T3 (l*32+k <- W3s[k,l]) then 32 out_low (W2s)
    Wrows3 = np.zeros((NPAIRS, N3), np.float64)
    Wrows2 = np.zeros((NPAIRS, KOUT), np.float64)
    for p, (i, j) in enumerate(PAIRS):
        if i < j:
            w3 = W3[i, j] + W3[j, i]   # [k, l]
            w2 = W2[i, j] + W2[j, i]   # [l]
        else:
            w3 = W3[i, i]
            w2 = W2[i, i]
        Wrows3[p] = w3.T.reshape(-1)   # col l*32+k
        Wrows2[p] = w2
    W3cat = np.zeros((5, 128, N3), np.float32)
    W2cat = np.zeros((5, 128, KOUT), np.float32)
    off = 0
    for c, pc in enumerate(CHUNKS):
        W3cat[c, :pc] = Wrows3[off:off + pc]
        W2cat[c, :pc] = Wrows2[off:off + pc]
        off += pc
    W1b = np.asarray(W1, np.float32).astype(BF16)       # [d, l]
    Sexp = np.zeros((5, 32, 128), np.float32)
    Srep = np.zeros((5, 32, 128), np.float32)
    off = 0
    for c, pc in enumerate(CHUNKS):
        for pp in range(pc):
            Sexp[c, I_P[off + pp], pp] = 1.0
            Srep[c, J_P[off + pp], pp] = 1.0
        off += pc
    return (W3cat.astype(BF16), W2cat.astype(BF16), W1b,
            Sexp.astype(BF16), Srep.astype(BF16))


def _build_module():
    nc = bacc.Bacc("TRN2", target_bir_lowering=False, debug=False,
                   enable_asserts=False)
    XBd = nc.dram_tensor("XB", [BLOC, D], BF, kind="ExternalInput").ap()
    XTd = nc.dram_tensor("XT", [D, BLOC], BF, kind="ExternalInput").ap()
    W3d = nc.dram_tensor("W3CAT", [5, 128, N3], BF, kind="ExternalInput").ap()
    W2d = nc.dram_tensor("W2CAT", [5, 128, KOUT], BF, kind="ExternalInput").ap()
    W1d = nc.dram_tensor("W1B", [D, KOUT], BF, kind="ExternalInput").ap()
    SEd = nc.dram_tensor("SEXP", [5, 32, 128], BF, kind="ExternalInput").ap()
    SRd = nc.dram_tensor("SREP", [5, 32, 128], BF, kind="ExternalInput").ap()
    OUTd = nc.dram_tensor("OUT", [BLOC, KOUT], F32, kind="ExternalOutput").ap()

    with ExitStack() as ctx:
        tc = ctx.enter_context(tile.TileContext(nc))
        consts = ctx.enter_context(tc.tile_pool(name="consts", bufs=1))
        xbpool = ctx.enter_context(tc.tile_pool(name="xbpool", bufs=3 * SUPER))
        xtpool = ctx.enter_context(tc.tile_pool(name="xtpool", bufs=4))
        repsb = ctx.enter_context(tc.tile_pool(name="repsb", bufs=4))
        zpool = ctx.enter_context(tc.tile_pool(name="zpool", bufs=3))
        spool = ctx.enter_context(tc.tile_pool(name="spool", bufs=3))
        upool = ctx.enter_context(tc.tile_pool(name="upool", bufs=3))
        rpool = ctx.enter_context(tc.tile_pool(name="rpool", bufs=3))
        opool = ctx.enter_context(tc.tile_pool(name="opool", bufs=4))
        bps = ctx.enter_context(tc.tile_pool(name="bps", bufs=2, space="PSUM"))
        t3ps = ctx.enter_context(tc.tile_pool(name="t3ps", bufs=2, space="PSUM"))
        olps = ctx.enter_context(tc.tile_pool(name="olps", bufs=2, space="PSUM"))

        # load constants
        w3_sb, w2_sb, se_sb, sr_sb = [], [], [], []
        for c in range(5):
            w3 = consts.tile([128, N3], BF, tag=f"w3_{c}")
            nc.sync.dma_start(out=w3, in_=W3d[c])
            w3_sb.append(w3)
            w2 = consts.tile([128, KOUT], BF, tag=f"w2_{c}")
            nc.sync.dma_start(out=w2, in_=W2d[c])
            w2_sb.append(w2)
            se = consts.tile([32, 128], BF, tag=f"se_{c}")
            nc.sync.dma_start(out=se, in_=SEd[c])
            se_sb.append(se)
            sr = consts.tile([32, 128], BF, tag=f"sr_{c}")
            nc.sync.dma_start(out=sr, in_=SRd[c])
            sr_sb.append(sr)
        w1_sb = consts.tile([D, KOUT], BF, tag="w1")
        nc.sync.dma_start(out=w1_sb, in_=W1d)

        for s in range(NSUPER):
            row0 = s * SUPER * P
            xt = xtpool.tile([D, SUPER * P], BF, tag="xt")
            nc.sync.dma_start(out=xt, in_=XTd[:, row0: row0 + SUPER * P])
            xbs = []
            for t in range(SUPER):
                xb = xbpool.tile([P, D], BF, tag="xb")
                nc.sync.dma_start(out=xb, in_=XBd[row0 + t * P: row0 + (t + 1) * P, :])
                xbs.append(xb)

            # build Z^T chunks for this supertile
            zs = []
            for c, pc in enumerate(CHUNKS):
                exp_ps = bps.tile([128, SUPER * P], F32, tag="bps")
                rep_ps = bps.tile([128, SUPER * P], F32, tag="bps")
                nc.tensor.matmul(exp_ps[:pc], se_sb[c][:, :pc], xt[0:D, :],
                                 start=True, stop=True)
                nc.tensor.matmul(rep_ps[:pc], sr_sb[c][:, :pc], xt[0:D, :],
                                 start=True, stop=True)
                rep_s = repsb.tile([128, SUPER * P], F32, tag="repsb")
                nc.scalar.copy(out=rep_s[:pc], in_=rep_ps[:pc])
                z = zpool.tile([128, SUPER * P], BF, tag=f"z{c}")
                nc.vector.tensor_mul(z[:pc], exp_ps[:pc], rep_s[:pc])
                zs.append(z)

            for t in range(SUPER):
                bsl = slice(t * P, (t + 1) * P)
                t3 = t3ps.tile([P, N3], F32, tag="t3")
                ol = olps.tile([P, KOUT], F32, tag="ol")
                for c, pc in enumerate(CHUNKS):
                    last = c == len(CHUNKS) - 1
                    nc.tensor.matmul(t3[:, 0:512], zs[c][:pc, bsl],
                                     w3_sb[c][:pc, 0:512],
                                     start=(c == 0), stop=last)
                    nc.tensor.matmul(t3[:, 512:1024], zs[c][:pc, bsl],
                                     w3_sb[c][:pc, 512:1024],
                                     start=(c == 0), stop=last)
                    nc.tensor.matmul(ol[:, :], zs[c][:pc, bsl],
                                     w2_sb[c][:pc, :],
                                     start=(c == 0), stop=False)
                # order-1: out_low += X^T.T @ W1
                nc.tensor.matmul(ol[:, :], xt[0:D, bsl], w1_sb,
                                 start=False, stop=True)

                staged = spool.tile([P, N3], BF, tag="staged")
                nc.scalar.copy(out=staged, in_=t3)
                u = upool.tile([P, N3], BF, tag="u")
                xk = xbs[t][:, 0:D].unsqueeze(1).broadcast_to([P, KOUT, D])
                nc.vector.tensor_mul(
                    u[:, :].rearrange("p (l k) -> p l k", k=D),
                    staged[:, :].rearrange("p (l k) -> p l k", k=D),
                    xk,
                )
                r = rpool.tile([P, KOUT], F32, tag="r")
                nc.vector.reduce_sum(
                    out=r, in_=u[:, :].rearrange("p (l k) -> p l k", k=D),
                    axis=mybir.AxisListType.X,
                )
                osb = opool.tile([P, KOUT], F32, tag="osb")
                nc.vector.tensor_add(osb, r, ol)
                nc.sync.dma_start(out=OUTd[row0 + t * P: row0 + (t + 1) * P, :],
                                  in_=osb)
    nc.compile()
    return nc


_CACHE = {}


def _get_module():
    if "nc" not in _CACHE:
        _CACHE["nc"] = _build_module()
    return _CACHE["nc"]


def kernel(X, W1, W2, W3, bias):
    X = np.ascontiguousarray(np.asarray(X, np.float32))
    bias = np.asarray(bias, np.float32)
    W3cat, W2cat, W1b, Sexp, Srep = _pack_weights(W1, W2, W3)

    nc = _get_module()
    Xb = X.astype(BF16)                      # [B, D] bf16 (single rounding point)
    XbT = np.ascontiguousarray(Xb.T)         # [D, B] bf16
    shards = Xb.reshape(NCORES, BLOC, D)
    in_maps = [
        {
            "XB": np.ascontiguousarray(shards[c]),
            "XT": np.ascontiguousarray(XbT[:, c * BLOC:(c + 1) * BLOC]),
            "W3CAT": W3cat,
            "W2CAT": W2cat,
            "W1B": W1b,
            "SEXP": Sexp,
            "SREP": Srep,
        }
        for c in range(NCORES)
    ]
    res = bass_utils.run_bass_kernel_spmd(nc, in_maps, core_ids=list(range(NCORES)))
    _CACHE["last_results"] = res
    out = np.concatenate([np.asarray(res.results[c]["OUT"]) for c in range(NCORES)], 0)
    return (out + bias.reshape(1, KOUT)).astype(np.float32)
